# revision 1
# baseline (speedup 1.0000x reference)
"""Bilateral blur (kornia 5x5, L1 color distance squared) on 8 TRN2 cores.

Data-parallel: one 1536x2048x3 fp32 image per NeuronCore. Residual form
  out = clip(ctr + (sum_o w_o * d_o) / den, 0, 1),  d_o = I(p+o) - I(p)
with the pair symmetry d_{-o}(p) = -d_{+o}(p-o): each unordered offset pair's
diff/weight planes are computed once and read at two alignments.

Findings baked in:
  - GpSimd activity contends with DVE via the shared SBUF port pair and slows
    every DVE op 30-90% -> all tensor work stays on the Vector engine.
  - fp16 (10-bit mantissa) gives near-fp32 accuracy for the residual form:
    emulated max abs err ~3.5e-5, max rel ~1.8e-3. The weight w is scaled by
    512 (bias += ln 512) to stay clear of fp16's subnormal floor; the scale
    cancels exactly in resid/den.
  - d must be produced by an fp32 subtract from fp32 pixels (relative-error
    regime); quantizing pixels first turns the error absolute and blows up
    the exponent accuracy of borderline weights.
  - t accumulation in fp16 is fine; 16-bit tensor_tensor runs at 2x.

Per-partition layouts (partition p owns img cols [16p-2, 16p+18)):
  T     (R+4) x (20px x 3ch)  fp32 interleaved
  d,|d| (R+2) x (3ch x 20px)  fp16 planar
  t,w   (R+2) x 20            fp16
  prod/resid  R x (3ch x 16)  fp16 planar
  den   R x 16                fp16    r32  R x 16  fp32
  stage R x 48                fp32 interleaved (px,ch) for DMA out
"""

import numpy as np
from contextlib import ExitStack

import concourse.bass as bass
import concourse.bacc as bacc
import concourse.mybir as mybir
import concourse.tile as tile
from concourse.bass_utils import run_bass_kernel_spmd
from bass_rust import VecI64Pair

F32 = mybir.dt.float32
F16 = mybir.dt.float16

H, W, C = 1536, 2048, 3
NCORES = 8
KS = 5
SIGMA_S = 1.0
SIGMA_R = 0.06
ROWE = 60
TCOL = 20
WSCALE = 512.0


def _constants():
    x = (np.arange(KS, dtype=np.float32) - KS // 2).astype(np.float32)
    g = np.exp(-0.5 * (x / np.float32(SIGMA_S)) ** 2).astype(np.float32)
    g = g / g.sum()
    space = np.outer(g, g).astype(np.float32)
    inv2sr2 = -0.5 / (SIGMA_R * SIGMA_R)
    return space, inv2sr2


SPACE, INV2SR2 = _constants()
A_SQ = float(np.sqrt(-INV2SR2))
S_CENTER = float(SPACE[2, 2])
PAIRS = [(0, 1), (0, 2)] + [(dy, dx) for dy in (1, 2) for dx in (-2, -1, 0, 1, 2)]
# pairs whose spatial weight is small enough that fp16-quantized inputs to the
# subtract only perturb the output at the ~2e-3 relative tail level
SMALL_S = {(0, 2), (1, -2), (1, 2), (2, -2), (2, -1), (2, 0), (2, 1), (2, 2)}
T16_SUBS = False  # True: 3.90ms but elem-rel tail 2.8e-2; False: safer 6.8e-3 tail


def _fview(ap2d, off, dims):
    v = ap2d.copy()
    v.offset = v.offset + off
    pdim = list(v.ap)[0]
    v.ap = VecI64Pair([list(pdim)] + [list(d) for d in dims])
    return v


def _dview(dram_ap, off, dims):
    v = dram_ap.copy()
    v.offset = v.offset + off
    v.ap = VecI64Pair([list(d) for d in dims])
    return v


def _pin_act_table_set():
    """Force every activation onto natural_log_exp_and_others (it holds all of
    Abs/Square/Exp/Ln), instead of walrus ping-ponging between exp_and_others
    and natural_log around each block's Ln (2 table reloads per block).
    Other sets are emptied but keep their positions so act_func_set_id
    indices stay aligned with act_info.json."""
    import concourse.hw_specs as hw_specs
    import concourse.bacc as bacc_mod
    orig = hw_specs.get_activation_tables
    if getattr(bacc_mod.get_activation_tables, "_pinned", False):
        return

    def patched(arch):
        t = dict(orig(arch))
        keep = "natural_log_exp_and_others"
        if keep in t:
            t = {k: (v if k == keep else set()) for k, v in t.items()}
        return t

    patched._pinned = True
    bacc_mod.get_activation_tables = patched


def build_nc(h=H, r=96):
    _pin_act_table_set()
    nb_blocks = h // r
    assert h % r == 0
    rowlen = W * C

    nc = bacc.Bacc("TRN2", target_bir_lowering=False, debug=False)
    img = nc.declare_dram_parameter("images", [h, W, C], F32, isOutput=False)
    out = nc.declare_dram_parameter("out", [h, W, C], F32, isOutput=True)
    img_a = img[:]
    out_a = out[:]

    with tile.TileContext(nc) as tc, ExitStack() as ctx:
        cpool = ctx.enter_context(tc.tile_pool(name="consts", bufs=1))
        tpool = ctx.enter_context(tc.tile_pool(name="input", bufs=2))
        rpool = ctx.enter_context(tc.tile_pool(name="resid", bufs=2))
        dnpool = ctx.enter_context(tc.tile_pool(name="den", bufs=2))
        rcpool = ctx.enter_context(tc.tile_pool(name="recip", bufs=1))
        dpool = ctx.enter_context(tc.tile_pool(name="diff", bufs=4))
        apool = ctx.enter_context(tc.tile_pool(name="absd", bufs=2))
        ttpool = ctx.enter_context(tc.tile_pool(name="tplane", bufs=2))
        wpool = ctx.enter_context(tc.tile_pool(name="wplane", bufs=3))
        ppool = ctx.enter_context(tc.tile_pool(name="prod", bufs=2))
        gpool = ctx.enter_context(tc.tile_pool(name="stage", bufs=2))
        t16pool = ctx.enter_context(tc.tile_pool(name="t16", bufs=2))

        consts = cpool.tile([128, 2 + len(PAIRS)], F32)
        ca = consts[:]
        nc.vector.memset(ca[:, 0:1], -1.0)
        nc.vector.memset(ca[:, 1:2], A_SQ)
        for i, (dy, dx) in enumerate(PAIRS):
            s = float(SPACE[dy + 2, dx + 2])
            nc.vector.memset(ca[:, 2 + i:3 + i], float(np.log(s * WSCALE)))
        neg1 = ca[:, 0:1]
        a_sq = ca[:, 1:2]

        def load_rows(ta, tile_r0, n, img_r0, sgn):
            if sgn < 0:
                for i in range(n):
                    load_rows(ta, tile_r0 + i, 1, img_r0 - i, 1)
                return
            rs = rowlen
            base = img_r0 * rowlen
            nc.sync.dma_start(
                out=_fview(ta[1:127], tile_r0 * ROWE, [[ROWE, n], [1, 60]]),
                in_=_dview(img_a, base + 42, [[48, 126], [rs, n], [1, 60]]),
            )
            nc.sync.dma_start(
                out=_fview(ta[0:1], tile_r0 * ROWE + 6, [[ROWE, n], [1, 54]]),
                in_=_dview(img_a, base + 0, [[0, 1], [rs, n], [1, 54]]),
            )
            for do, so in ((0, 6), (3, 3)):
                nc.sync.dma_start(
                    out=_fview(ta[0:1], tile_r0 * ROWE + do, [[ROWE, n], [1, 3]]),
                    in_=_dview(img_a, base + so, [[0, 1], [rs, n], [1, 3]]),
                )
            nc.sync.dma_start(
                out=_fview(ta[127:128], tile_r0 * ROWE, [[ROWE, n], [1, 54]]),
                in_=_dview(img_a, base + 6090, [[0, 1], [rs, n], [1, 54]]),
            )
            for do, so in ((54, 6138), (57, 6135)):
                nc.sync.dma_start(
                    out=_fview(ta[127:128], tile_r0 * ROWE + do, [[ROWE, n], [1, 3]]),
                    in_=_dview(img_a, base + so, [[0, 1], [rs, n], [1, 3]]),
                )

        for b in range(nb_blocks):
            r0 = b * r
            tin = tpool.tile([128, (r + 4) * ROWE], F32)
            ta = tin[:]
            if nb_blocks == 1:
                load_rows(ta, 2, r, 0, 1)
                load_rows(ta, 0, 2, 2, -1)
                load_rows(ta, r + 2, 2, h - 2, -1)
            elif b == 0:
                load_rows(ta, 2, r + 2, 0, 1)
                load_rows(ta, 0, 2, 2, -1)
            elif b == nb_blocks - 1:
                load_rows(ta, 0, r + 2, r0 - 2, 1)
                load_rows(ta, r + 2, 2, h - 2, -1)
            else:
                load_rows(ta, 0, r + 4, r0 - 2, 1)

            t16 = None
            if T16_SUBS:
                # fp16 planar copy of T for the small-weight pairs' subtracts
                t16 = t16pool.tile([128, (r + 4) * ROWE], F16)
                nc.scalar.activation(
                    _fview(t16[:], 0, [[ROWE, r + 4], [TCOL, 3], [1, 20]]),
                    _fview(ta, 0, [[ROWE, r + 4], [1, 3], [3, 20]]),
                    mybir.ActivationFunctionType.Copy)

            resid = rpool.tile([128, r * 48], F16)
            den = dnpool.tile([128, r * 16], F16)
            ra = resid[:]
            da = den[:]
            first_resid = [True]
            first_den = [True]

            for i, (dy, dx) in enumerate(PAIRS):
                qr0 = -dy
                nqr = r + dy
                qc0 = -max(dx, 0)
                col_lo = qc0 + 2
                col_e = col_lo & ~1
                nqc = 16 + abs(dx) + (col_lo - col_e)
                ri0 = qr0 + 2

                dt_ = dpool.tile([128, (r + 2) * ROWE], F16)
                ad_ = apool.tile([128, (r + 2) * ROWE], F16)
                tt_ = ttpool.tile([128, (r + 2) * TCOL], F16)
                tw_ = wpool.tile([128, (r + 2) * TCOL], F16)
                dv, av, tv, wv = dt_[:], ad_[:], tt_[:], tw_[:]

                d_out = _fview(dv, ri0 * ROWE + col_e, [[ROWE, nqr], [TCOL, 3], [1, nqc]])
                if T16_SUBS and (dy, dx) in SMALL_S:
                    # fp16 2x subtract from the planar fp16 copy
                    nc.vector.tensor_tensor(
                        d_out,
                        _fview(t16[:], (ri0 + dy) * ROWE + col_e + dx,
                               [[ROWE, nqr], [TCOL, 3], [1, nqc]]),
                        _fview(t16[:], ri0 * ROWE + col_e,
                               [[ROWE, nqr], [TCOL, 3], [1, nqc]]),
                        mybir.AluOpType.subtract)
                else:
                    nc.vector.tensor_tensor(
                        d_out,
                        _fview(ta, (ri0 + dy) * ROWE + (col_e + dx) * 3,
                               [[ROWE, nqr], [1, 3], [3, nqc]]),
                        _fview(ta, ri0 * ROWE + col_e * 3,
                               [[ROWE, nqr], [1, 3], [3, nqc]]),
                        mybir.AluOpType.subtract)
                nc.scalar.activation(
                    _fview(av, ri0 * ROWE + col_e, [[ROWE, nqr], [TCOL, 3], [1, nqc]]),
                    d_out, mybir.ActivationFunctionType.Abs)
                tq = _fview(tv, ri0 * TCOL + col_e, [[TCOL, nqr], [1, nqc]])
                nc.vector.tensor_tensor(
                    tq,
                    _fview(av, ri0 * ROWE + 0 * TCOL + col_e, [[ROWE, nqr], [1, nqc]]),
                    _fview(av, ri0 * ROWE + 1 * TCOL + col_e, [[ROWE, nqr], [1, nqc]]),
                    mybir.AluOpType.add)
                nc.vector.tensor_tensor(
                    tq, tq,
                    _fview(av, ri0 * ROWE + 2 * TCOL + col_e, [[ROWE, nqr], [1, nqc]]),
                    mybir.AluOpType.add)
                nc.scalar.activation(tq, tq, mybir.ActivationFunctionType.Square,
                                     scale=a_sq)
                wq = _fview(wv, ri0 * TCOL + col_e, [[TCOL, nqr], [1, nqc]])
                nc.scalar.activation(wq, tq, mybir.ActivationFunctionType.Exp,
                                     bias=ca[:, 2 + i:3 + i], scale=neg1)

                # pi(q) = w(q)*d(q) once on the extended grid, in place over d;
                # both directions' contributions are slices of pi:
                #   resid += pi(p) ; resid -= pi(p-o)   (bit-identical to the
                # per-direction products, ~45% fewer multiply elements)
                for ch in range(3):
                    dchq = _fview(dv, ri0 * ROWE + ch * TCOL + col_e,
                                  [[ROWE, nqr], [1, nqc]])
                    nc.vector.tensor_tensor(
                        dchq, dchq,
                        _fview(wv, ri0 * TCOL + col_e, [[TCOL, nqr], [1, nqc]]),
                        mybir.AluOpType.mult)
                for sg in (1, -1):
                    ri, ci = (2, 2) if sg == 1 else (2 - dy, 2 - dx)
                    pi_sl = _fview(dv, ri * ROWE + ci, [[ROWE, r], [TCOL, 3], [1, 16]])
                    if first_resid[0]:
                        assert sg == 1
                        nc.vector.tensor_copy(
                            _fview(ra, 0, [[48, r], [16, 3], [1, 16]]), pi_sl)
                        first_resid[0] = False
                    else:
                        nc.vector.tensor_tensor(
                            _fview(ra, 0, [[48, r], [16, 3], [1, 16]]),
                            _fview(ra, 0, [[48, r], [16, 3], [1, 16]]),
                            pi_sl,
                            mybir.AluOpType.add if sg == 1 else mybir.AluOpType.subtract)
                    wslice = _fview(wv, ri * TCOL + ci, [[TCOL, r], [1, 16]])
                    if first_den[0]:
                        nc.vector.tensor_scalar_add(da, wslice, S_CENTER * WSCALE)
                        first_den[0] = False
                    else:
                        nc.vector.tensor_tensor(da, da, wslice, mybir.AluOpType.add)

            # 1/den (x WSCALE, cancels): r32 = exp(-ln(den))
            r32 = rcpool.tile([128, r * 16], F32)
            rca = r32[:]
            nc.scalar.activation(rca, da, mybir.ActivationFunctionType.Ln)
            nc.scalar.activation(rca, rca, mybir.ActivationFunctionType.Exp,
                                 scale=neg1)
            stage = gpool.tile([128, r * 48], F32)
            sa = stage[:]
            for ch in range(3):
                nc.vector.tensor_tensor(
                    _fview(sa, ch, [[48, r], [3, 16]]),
                    _fview(ra, ch * 16, [[48, r], [1, 16]]),
                    _fview(rca, 0, [[16, r], [1, 16]]),
                    mybir.AluOpType.mult)
            nc.vector.tensor_tensor(
                sa, sa, _fview(ta, 2 * ROWE + 6, [[ROWE, r], [1, 48]]),
                mybir.AluOpType.add)
            nc.vector.tensor_scalar(sa, sa, 0.0, 1.0,
                                    mybir.AluOpType.max, mybir.AluOpType.min)
            nc.sync.dma_start(
                out=_dview(out_a, r0 * rowlen, [[48, 128], [rowlen, r], [1, 48]]),
                in_=_fview(sa, 0, [[48, r], [1, 48]]),
            )
    nc.finalize()
    return nc


_CACHE = {}


def _get_nc(h=H, r=96):
    key = (h, r)
    if key not in _CACHE:
        _CACHE[key] = build_nc(h, r)
    return _CACHE[key]


TRACE = False
LAST_RESULT = None


def kernel(images: np.ndarray) -> np.ndarray:
    global LAST_RESULT
    assert images.shape == (NCORES, H, W, C), images.shape
    nc = _get_nc()
    in_maps = [{"images": np.ascontiguousarray(images[i], dtype=np.float32)}
               for i in range(NCORES)]
    res = run_bass_kernel_spmd(nc, in_maps, core_ids=list(range(NCORES)),
                               trace=TRACE)
    LAST_RESULT = res
    return np.stack([res.results[i]["out"] for i in range(NCORES)], axis=0)



# revision 2
# speedup vs baseline: 1.0404x; 1.0404x over previous
"""Bilateral blur (kornia 5x5, L1 color distance squared) on 8 TRN2 cores — v3.

v3 over v2 (PE/PSUM accumulation): the per-pair subtracts read a planar fp16
copy of the pixel block (one Act convert per block) so they run at DVE 2x
instead of the fp32 1x; the abs for ABS_ON_DVE pairs moves to the DVE as a
tensor_scalar(abs_max, 0) which runs at 4x, balancing the two engines; and
the final combine does the center-add in planar fp16 space, fusing the
interleave + fp32 cast into the single Act copy before the clip.
"""

import numpy as np
from contextlib import ExitStack

import concourse.bass as bass
import concourse.bacc as bacc
import concourse.mybir as mybir
import concourse.tile as tile
from concourse.bass_utils import run_bass_kernel_spmd
from concourse.masks import make_identity
from bass_rust import VecI64Pair

F32 = mybir.dt.float32
F16 = mybir.dt.float16

H, W, C = 1536, 2048, 3
NCORES = 8
KS = 5
SIGMA_S = 1.0
SIGMA_R = 0.06
ROWE = 60
TCOL = 20
WSCALE = 512.0
R = 64


def _constants():
    x = (np.arange(KS, dtype=np.float32) - KS // 2).astype(np.float32)
    g = np.exp(-0.5 * (x / np.float32(SIGMA_S)) ** 2).astype(np.float32)
    g = g / g.sum()
    space = np.outer(g, g).astype(np.float32)
    inv2sr2 = -0.5 / (SIGMA_R * SIGMA_R)
    return space, inv2sr2


SPACE, INV2SR2 = _constants()
A_SQ = float(np.sqrt(-INV2SR2))
S_CENTER = float(SPACE[2, 2])
PAIRS = [(0, 1), (0, 2)] + [(dy, dx) for dy in (1, 2) for dx in (-2, -1, 0, 1, 2)]
NDIR = 2 * len(PAIRS)  # 24 accumulation directions
# pairs whose fp32 subtract is kept (empty: all subs read the fp16 copy)
F32_SUBS = set()
# pairs whose |d| runs on the DVE (int16 bitcast + bitwise_and 0x7fff @4x)
ABS_ON_DVE = set()
# pairs whose square runs on the DVE (tensor_tensor t*t @2x) instead of Act
SQ_ON_DVE = {(1, 0), (2, 0), (0, 1)}


def _fview(ap2d, off, dims):
    v = ap2d.copy()
    v.offset = v.offset + off
    pdim = list(v.ap)[0]
    v.ap = VecI64Pair([list(pdim)] + [list(d) for d in dims])
    return v


def _dview(dram_ap, off, dims):
    v = dram_ap.copy()
    v.offset = v.offset + off
    v.ap = VecI64Pair([list(d) for d in dims])
    return v


def _pin_act_table_set():
    """Force every activation onto natural_log_exp_and_others (it holds all of
    Abs/Square/Exp/Ln) to avoid per-block table reloads."""
    import concourse.hw_specs as hw_specs
    import concourse.bacc as bacc_mod
    orig = hw_specs.get_activation_tables
    if getattr(bacc_mod.get_activation_tables, "_pinned", False):
        return

    def patched(arch):
        t = dict(orig(arch))
        keep = "natural_log_exp_and_others"
        if keep in t:
            t = {k: (v if k == keep else set()) for k, v in t.items()}
        return t

    patched._pinned = True
    bacc_mod.get_activation_tables = patched


def build_nc(h=H, r=R):
    _pin_act_table_set()
    nb_blocks = h // r
    assert h % r == 0
    assert r % 8 == 0 and r % 32 == 0
    rowlen = W * C

    nc = bacc.Bacc("TRN2", target_bir_lowering=False, debug=False)
    img = nc.declare_dram_parameter("images", [h, W, C], F32, isOutput=False)
    out = nc.declare_dram_parameter("out", [h, W, C], F32, isOutput=True)
    img_a = img[:]
    out_a = out[:]

    with tile.TileContext(nc) as tc, ExitStack() as ctx:
        cpool = ctx.enter_context(tc.tile_pool(name="consts", bufs=1))
        tpool = ctx.enter_context(tc.tile_pool(name="input", bufs=2))
        dpool = ctx.enter_context(tc.tile_pool(name="diff", bufs=4))
        apool = ctx.enter_context(tc.tile_pool(name="absd", bufs=2))
        ttpool = ctx.enter_context(tc.tile_pool(name="tplane", bufs=2))
        wpool = ctx.enter_context(tc.tile_pool(name="wplane", bufs=3))
        pspool = ctx.enter_context(
            tc.tile_pool(name="acc", bufs=1, space="PSUM"))
        t16pool = ctx.enter_context(tc.tile_pool(name="t16", bufs=2))
        rspool = ctx.enter_context(tc.tile_pool(name="rs16", bufs=2))
        sppool = ctx.enter_context(tc.tile_pool(name="stpl", bufs=2))
        gpool = ctx.enter_context(tc.tile_pool(name="stage", bufs=2))
        rcpool = ctx.enter_context(tc.tile_pool(name="recip", bufs=2))
        rrpool = ctx.enter_context(tc.tile_pool(name="rr", bufs=2))

        consts = cpool.tile([128, 3 + len(PAIRS)], F32)
        ca = consts[:]
        nc.vector.memset(ca[:, 0:1], -1.0)
        nc.vector.memset(ca[:, 1:2], A_SQ)
        for i, (dy, dx) in enumerate(PAIRS):
            s = float(SPACE[dy + 2, dx + 2])
            nc.vector.memset(ca[:, 2 + i:3 + i], float(np.log(s * WSCALE)))
        cbias_col = 2 + len(PAIRS)
        nc.vector.memset(ca[:, cbias_col:cbias_col + 1],
                         float(S_CENTER * WSCALE))
        neg1 = ca[:, 0:1]
        a_sq = ca[:, 1:2]
        den_c = ca[:, cbias_col:cbias_col + 1]

        posI = cpool.tile([128, 128], F16)
        negI = cpool.tile([128, 128], F16)
        make_identity(nc, posI[:])
        nc.vector.tensor_scalar_mul(negI[:], posI[:], -1.0)
        posIa, negIa = posI[:], negI[:]

        def load_rows(ta, tile_r0, n, img_r0, sgn):
            if sgn < 0:
                for i in range(n):
                    load_rows(ta, tile_r0 + i, 1, img_r0 - i, 1)
                return
            rs = rowlen
            base = img_r0 * rowlen
            nc.sync.dma_start(
                out=_fview(ta[1:127], tile_r0 * ROWE, [[ROWE, n], [1, 60]]),
                in_=_dview(img_a, base + 42, [[48, 126], [rs, n], [1, 60]]),
            )
            nc.sync.dma_start(
                out=_fview(ta[0:1], tile_r0 * ROWE + 6, [[ROWE, n], [1, 54]]),
                in_=_dview(img_a, base + 0, [[0, 1], [rs, n], [1, 54]]),
            )
            for do, so in ((0, 6), (3, 3)):
                nc.sync.dma_start(
                    out=_fview(ta[0:1], tile_r0 * ROWE + do, [[ROWE, n], [1, 3]]),
                    in_=_dview(img_a, base + so, [[0, 1], [rs, n], [1, 3]]),
                )
            nc.sync.dma_start(
                out=_fview(ta[127:128], tile_r0 * ROWE, [[ROWE, n], [1, 54]]),
                in_=_dview(img_a, base + 6090, [[0, 1], [rs, n], [1, 54]]),
            )
            for do, so in ((54, 6138), (57, 6135)):
                nc.sync.dma_start(
                    out=_fview(ta[127:128], tile_r0 * ROWE + do, [[ROWE, n], [1, 3]]),
                    in_=_dview(img_a, base + so, [[0, 1], [rs, n], [1, 3]]),
                )

        for b in range(nb_blocks):
            r0 = b * r
            tin = tpool.tile([128, (r + 4) * ROWE], F32)
            ta = tin[:]
            if nb_blocks == 1:
                load_rows(ta, 2, r, 0, 1)
                load_rows(ta, 0, 2, 2, -1)
                load_rows(ta, r + 2, 2, h - 2, -1)
            elif b == 0:
                load_rows(ta, 2, r + 2, 0, 1)
                load_rows(ta, 0, 2, 2, -1)
            elif b == nb_blocks - 1:
                load_rows(ta, 0, r + 2, r0 - 2, 1)
                load_rows(ta, r + 2, 2, h - 2, -1)
            else:
                load_rows(ta, 0, r + 4, r0 - 2, 1)

            # planar fp16 copy of the pixel block: subs at 2x, ctr-add at 2x
            t16 = t16pool.tile([128, (r + 4) * ROWE], F16)
            t16a = t16[:]
            nc.scalar.activation(
                _fview(t16a, 0, [[ROWE, r + 4], [TCOL, 3], [1, 20]]),
                _fview(ta, 0, [[ROWE, r + 4], [1, 3], [3, 20]]),
                mybir.ActivationFunctionType.Copy)

            # PSUM rows are 64 fp32 wide: [48 resid | 16 den]. Each 8-row
            # resid chunk is then exactly one 2048B PSUM zero-region, so
            # start_tensor_calc's region-granular zeroing stays aligned;
            # the den stripes ride in the same regions (no separate start).
            acc = pspool.tile([128, r * 64], F32)
            aa_ps = acc[:]

            dir_idx = 0
            for i, (dy, dx) in enumerate(PAIRS):
                qr0 = -dy
                nqr = r + dy
                qc0 = -max(dx, 0)
                col_lo = qc0 + 2
                col_e = col_lo & ~1
                nqc = 16 + abs(dx) + (col_lo - col_e)
                ri0 = qr0 + 2

                dt_ = dpool.tile([128, (r + 2) * ROWE], F16)
                ad_ = apool.tile([128, (r + 2) * ROWE], F16)
                tt_ = ttpool.tile([128, (r + 2) * TCOL], F16)
                tw_ = wpool.tile([128, (r + 2) * TCOL], F16)
                dv, av, tv, wv = dt_[:], ad_[:], tt_[:], tw_[:]

                d_out = _fview(dv, ri0 * ROWE + col_e, [[ROWE, nqr], [TCOL, 3], [1, nqc]])
                if (dy, dx) in F32_SUBS:
                    nc.vector.tensor_tensor(
                        d_out,
                        _fview(ta, (ri0 + dy) * ROWE + (col_e + dx) * 3,
                               [[ROWE, nqr], [1, 3], [3, nqc]]),
                        _fview(ta, ri0 * ROWE + col_e * 3,
                               [[ROWE, nqr], [1, 3], [3, nqc]]),
                        mybir.AluOpType.subtract)
                else:
                    nc.vector.tensor_tensor(
                        d_out,
                        _fview(t16a, (ri0 + dy) * ROWE + col_e + dx,
                               [[ROWE, nqr], [TCOL, 3], [1, nqc]]),
                        _fview(t16a, ri0 * ROWE + col_e,
                               [[ROWE, nqr], [TCOL, 3], [1, nqc]]),
                        mybir.AluOpType.subtract)
                a_out = _fview(av, ri0 * ROWE + col_e,
                               [[ROWE, nqr], [TCOL, 3], [1, nqc]])
                if (dy, dx) in ABS_ON_DVE:
                    # |x| on fp16 = clear the sign bit; int16 view keeps the
                    # 2-byte packed 4x DVE mode
                    nc.vector.tensor_scalar(
                        a_out.bitcast(mybir.dt.int16),
                        d_out.bitcast(mybir.dt.int16),
                        0x7FFF, None, mybir.AluOpType.bitwise_and)
                else:
                    nc.scalar.activation(
                        a_out, d_out, mybir.ActivationFunctionType.Abs)
                tq = _fview(tv, ri0 * TCOL + col_e, [[TCOL, nqr], [1, nqc]])
                nc.vector.tensor_tensor(
                    tq,
                    _fview(av, ri0 * ROWE + 0 * TCOL + col_e, [[ROWE, nqr], [1, nqc]]),
                    _fview(av, ri0 * ROWE + 1 * TCOL + col_e, [[ROWE, nqr], [1, nqc]]),
                    mybir.AluOpType.add)
                nc.vector.tensor_tensor(
                    tq, tq,
                    _fview(av, ri0 * ROWE + 2 * TCOL + col_e, [[ROWE, nqr], [1, nqc]]),
                    mybir.AluOpType.add)
                wq = _fview(wv, ri0 * TCOL + col_e, [[TCOL, nqr], [1, nqc]])
                if (dy, dx) in SQ_ON_DVE:
                    # u = t*t on DVE; fold a^2 into the exp scale
                    nc.vector.tensor_tensor(tq, tq, tq, mybir.AluOpType.mult)
                    nc.scalar.activation(wq, tq,
                                         mybir.ActivationFunctionType.Exp,
                                         bias=ca[:, 2 + i:3 + i],
                                         scale=float(INV2SR2))
                else:
                    nc.scalar.activation(tq, tq,
                                         mybir.ActivationFunctionType.Square,
                                         scale=a_sq)
                    nc.scalar.activation(wq, tq,
                                         mybir.ActivationFunctionType.Exp,
                                         bias=ca[:, 2 + i:3 + i], scale=neg1)

                # pi = w*d in place over d on the extended grid
                for ch in range(3):
                    dchq = _fview(dv, ri0 * ROWE + ch * TCOL + col_e,
                                  [[ROWE, nqr], [1, nqc]])
                    nc.vector.tensor_tensor(
                        dchq, dchq,
                        _fview(wv, ri0 * TCOL + col_e, [[TCOL, nqr], [1, nqc]]),
                        mybir.AluOpType.mult)

                # PE accumulation: resid += pi(p); resid -= pi(p-o);
                #                  den += w(p) + w(p-o)
                # Emission order: +I resid (starts each region on dir 0),
                # +I den(x2), -I resid (stops each region on the last pair).
                rc_pos = (2, 2)
                rc_neg = (2 - dy, 2 - dx)
                st = dir_idx == 0
                sp = dir_idx == NDIR - 2
                for j in range(r // 8):
                    nc.tensor.matmul(
                        _fview(aa_ps, (8 * j) * 64, [[64, 8], [16, 3], [1, 16]]),
                        posIa,
                        _fview(dv, (rc_pos[0] + 8 * j) * ROWE + rc_pos[1],
                               [[ROWE, 8], [TCOL, 3], [1, 16]]),
                        start=st, stop=False)
                for (ri2, ci2) in (rc_pos, rc_neg):
                    for j in range(r // 32):
                        nc.tensor.matmul(
                            _fview(aa_ps, (32 * j) * 64 + 48,
                                   [[64, 32], [1, 16]]),
                            posIa,
                            _fview(wv, (ri2 + 32 * j) * TCOL + ci2,
                                   [[TCOL, 32], [1, 16]]),
                            start=False, stop=False)
                for j in range(r // 8):
                    nc.tensor.matmul(
                        _fview(aa_ps, (8 * j) * 64, [[64, 8], [16, 3], [1, 16]]),
                        negIa,
                        _fview(dv, (rc_neg[0] + 8 * j) * ROWE + rc_neg[1],
                               [[ROWE, 8], [TCOL, 3], [1, 16]]),
                        start=False, stop=sp)
                dir_idx += 2

            # Final combine: out = clip(ctr + resid * (1/(den+c)), 0, 1)
            rs_ = rspool.tile([128, r * 48], F16)
            rsa = rs_[:]
            nc.scalar.activation(
                _fview(rsa, 0, [[48, r], [16, 3], [1, 16]]),
                _fview(aa_ps, 0, [[64, r], [16, 3], [1, 16]]),
                mybir.ActivationFunctionType.Copy)
            r32 = rcpool.tile([128, r * 16], F32)
            rca = r32[:]
            nc.scalar.activation(
                rca, _fview(aa_ps, 48, [[64, r], [1, 16]]),
                mybir.ActivationFunctionType.Ln, bias=den_c)
            rr_ = rrpool.tile([128, r * 16], F16)
            rra = rr_[:]
            nc.scalar.activation(rra, rca, mybir.ActivationFunctionType.Exp,
                                 scale=neg1)
            spl_ = sppool.tile([128, r * 48], F16)
            spla = spl_[:]
            for ch in range(3):
                nc.vector.tensor_tensor(
                    _fview(spla, ch * 16, [[48, r], [1, 16]]),
                    _fview(rsa, ch * 16, [[48, r], [1, 16]]),
                    _fview(rra, 0, [[16, r], [1, 16]]),
                    mybir.AluOpType.mult)
            # center add in planar fp16 (2x), then one Act copy does
            # planar->interleaved AND fp16->fp32
            nc.vector.tensor_tensor(
                _fview(spla, 0, [[48, r], [16, 3], [1, 16]]),
                _fview(spla, 0, [[48, r], [16, 3], [1, 16]]),
                _fview(t16a, 2 * ROWE + 2, [[ROWE, r], [TCOL, 3], [1, 16]]),
                mybir.AluOpType.add)
            stage = gpool.tile([128, r * 48], F32)
            sa = stage[:]
            nc.scalar.activation(
                _fview(sa, 0, [[48, r], [1, 3], [3, 16]]),
                _fview(spla, 0, [[48, r], [16, 3], [1, 16]]),
                mybir.ActivationFunctionType.Copy)
            nc.vector.tensor_scalar(sa, sa, 0.0, 1.0,
                                    mybir.AluOpType.max, mybir.AluOpType.min)
            nc.sync.dma_start(
                out=_dview(out_a, r0 * rowlen, [[48, 128], [rowlen, r], [1, 48]]),
                in_=_fview(sa, 0, [[48, r], [1, 48]]),
            )
    nc.finalize()
    return nc


_CACHE = {}


def _get_nc(h=H, r=R):
    key = (h, r)
    if key not in _CACHE:
        _CACHE[key] = build_nc(h, r)
    return _CACHE[key]


TRACE = False
LAST_RESULT = None


def kernel(images: np.ndarray) -> np.ndarray:
    global LAST_RESULT
    assert images.shape == (NCORES, H, W, C), images.shape
    nc = _get_nc()
    in_maps = [{"images": np.ascontiguousarray(images[i], dtype=np.float32)}
               for i in range(NCORES)]
    res = run_bass_kernel_spmd(nc, in_maps, core_ids=list(range(NCORES)),
                               trace=TRACE)
    LAST_RESULT = res
    return np.stack([res.results[i]["out"] for i in range(NCORES)], axis=0)


# revision 4
# speedup vs baseline: 1.0438x; 1.0033x over previous
"""Bilateral blur (kornia 5x5, L1 color distance squared) on 8 TRN2 cores.

v6 (2.06ms vs the 3.84ms v1 baseline; rel err 6.6e-4 vs gate 2e-2):
  - All 48 resid/den accumulates per block run on the otherwise-idle PE as
    identity-stationary fp16 matmuls accumulating into PSUM fp32. PSUM rows
    are 64 fp32 wide ([48 resid | 16 den]) so each 8-row resid chunk is
    exactly one 2048B PSUM zero-region - start_tensor_calc zeroing is
    region-granular and misaligned chunks corrupt earlier accumulations.
  - Subtracts read a planar fp16 copy of the pixel block (one Act convert
    per block) so they run at DVE 2x instead of fp32 1x. Accuracy headroom
    is large (absmax 6.6e-4).
  - Engine balance: DVE and Act both land at ~72us/block (84% busy each).
    ABS_ON_DVE pairs use int16-bitcast bitwise_and 0x7fff (a 4x
    tensor_scalar; the abs_max ALU op fails the walrus ISA check), and
    SQ_ON_DVE pairs square t on the DVE with a^2 folded into the exp scale.
  - The pi products + matmuls are emitted one pair late ("back") so the
    in-order DVE stream never waits on the same pair's exp. A deeper 3-stage
    skew REGRESSED 20% (all engines' active time inflates - SBUF port
    contention under higher engine concurrency), so keep the 1-stage defer.
  - The center den term folds into the Ln bias of the reciprocal; the
    center pixel add happens in planar fp16, and one Act copy fuses
    planar->interleaved with the fp16->fp32 cast before the clip.
"""

import numpy as np
from contextlib import ExitStack

import concourse.bass as bass
import concourse.bacc as bacc
import concourse.mybir as mybir
import concourse.tile as tile
from concourse.bass_utils import run_bass_kernel_spmd
from concourse.masks import make_identity
from bass_rust import VecI64Pair

F32 = mybir.dt.float32
F16 = mybir.dt.float16

H, W, C = 1536, 2048, 3
NCORES = 8
KS = 5
SIGMA_S = 1.0
SIGMA_R = 0.06
ROWE = 60
TCOL = 20
WSCALE = 512.0
R = 64


def _constants():
    x = (np.arange(KS, dtype=np.float32) - KS // 2).astype(np.float32)
    g = np.exp(-0.5 * (x / np.float32(SIGMA_S)) ** 2).astype(np.float32)
    g = g / g.sum()
    space = np.outer(g, g).astype(np.float32)
    inv2sr2 = -0.5 / (SIGMA_R * SIGMA_R)
    return space, inv2sr2


SPACE, INV2SR2 = _constants()
A_SQ = float(np.sqrt(-INV2SR2))
S_CENTER = float(SPACE[2, 2])
PAIRS = [(0, 1), (0, 2)] + [(dy, dx) for dy in (1, 2) for dx in (-2, -1, 0, 1, 2)]
NDIR = 2 * len(PAIRS)  # 24 accumulation directions
# pairs whose fp32 subtract is kept (empty: all subs read the fp16 copy)
F32_SUBS = set()
# pairs whose |d| runs on the DVE (int16 bitcast + bitwise_and 0x7fff @4x)
ABS_ON_DVE = {(2, -1), (2, 1)}
# pairs whose square runs on the DVE (tensor_tensor t*t @2x) instead of Act
SQ_ON_DVE = {(1, 0), (2, 0), (1, 1)}


def _fview(ap2d, off, dims):
    v = ap2d.copy()
    v.offset = v.offset + off
    pdim = list(v.ap)[0]
    v.ap = VecI64Pair([list(pdim)] + [list(d) for d in dims])
    return v


def _dview(dram_ap, off, dims):
    v = dram_ap.copy()
    v.offset = v.offset + off
    v.ap = VecI64Pair([list(d) for d in dims])
    return v


def _pin_act_table_set():
    """Force every activation onto natural_log_exp_and_others (it holds all of
    Abs/Square/Exp/Ln) to avoid per-block table reloads."""
    import concourse.hw_specs as hw_specs
    import concourse.bacc as bacc_mod
    orig = hw_specs.get_activation_tables
    if getattr(bacc_mod.get_activation_tables, "_pinned", False):
        return

    def patched(arch):
        t = dict(orig(arch))
        keep = "natural_log_exp_and_others"
        if keep in t:
            t = {k: (v if k == keep else set()) for k, v in t.items()}
        return t

    patched._pinned = True
    bacc_mod.get_activation_tables = patched


def build_nc(h=H, r=R):
    _pin_act_table_set()
    nb_blocks = h // r
    assert h % r == 0
    assert r % 8 == 0 and r % 32 == 0
    rowlen = W * C

    nc = bacc.Bacc("TRN2", target_bir_lowering=False, debug=False)
    img = nc.declare_dram_parameter("images", [h, W, C], F32, isOutput=False)
    out = nc.declare_dram_parameter("out", [h, W, C], F32, isOutput=True)
    img_a = img[:]
    out_a = out[:]

    with tile.TileContext(nc) as tc, ExitStack() as ctx:
        cpool = ctx.enter_context(tc.tile_pool(name="consts", bufs=1))
        tpool = ctx.enter_context(tc.tile_pool(name="input", bufs=2))
        dpool = ctx.enter_context(tc.tile_pool(name="diff", bufs=4))
        apool = ctx.enter_context(tc.tile_pool(name="absd", bufs=2))
        ttpool = ctx.enter_context(tc.tile_pool(name="tplane", bufs=2))
        wpool = ctx.enter_context(tc.tile_pool(name="wplane", bufs=3))
        pspool = ctx.enter_context(
            tc.tile_pool(name="acc", bufs=1, space="PSUM"))
        t16pool = ctx.enter_context(tc.tile_pool(name="t16", bufs=2))
        rspool = ctx.enter_context(tc.tile_pool(name="rs16", bufs=2))
        sppool = ctx.enter_context(tc.tile_pool(name="stpl", bufs=2))
        gpool = ctx.enter_context(tc.tile_pool(name="stage", bufs=2))
        rcpool = ctx.enter_context(tc.tile_pool(name="recip", bufs=2))
        rrpool = ctx.enter_context(tc.tile_pool(name="rr", bufs=2))

        consts = cpool.tile([128, 3 + len(PAIRS)], F32)
        ca = consts[:]
        nc.vector.memset(ca[:, 0:1], -1.0)
        nc.vector.memset(ca[:, 1:2], A_SQ)
        for i, (dy, dx) in enumerate(PAIRS):
            s = float(SPACE[dy + 2, dx + 2])
            nc.vector.memset(ca[:, 2 + i:3 + i], float(np.log(s * WSCALE)))
        cbias_col = 2 + len(PAIRS)
        nc.vector.memset(ca[:, cbias_col:cbias_col + 1],
                         float(S_CENTER * WSCALE))
        neg1 = ca[:, 0:1]
        a_sq = ca[:, 1:2]
        den_c = ca[:, cbias_col:cbias_col + 1]

        posI = cpool.tile([128, 128], F16)
        negI = cpool.tile([128, 128], F16)
        make_identity(nc, posI[:])
        nc.vector.tensor_scalar_mul(negI[:], posI[:], -1.0)
        posIa, negIa = posI[:], negI[:]

        def load_rows(ta, tile_r0, n, img_r0, sgn):
            if sgn < 0:
                for i in range(n):
                    load_rows(ta, tile_r0 + i, 1, img_r0 - i, 1)
                return
            rs = rowlen
            base = img_r0 * rowlen
            nc.sync.dma_start(
                out=_fview(ta[1:127], tile_r0 * ROWE, [[ROWE, n], [1, 60]]),
                in_=_dview(img_a, base + 42, [[48, 126], [rs, n], [1, 60]]),
            )
            nc.sync.dma_start(
                out=_fview(ta[0:1], tile_r0 * ROWE + 6, [[ROWE, n], [1, 54]]),
                in_=_dview(img_a, base + 0, [[0, 1], [rs, n], [1, 54]]),
            )
            for do, so in ((0, 6), (3, 3)):
                nc.sync.dma_start(
                    out=_fview(ta[0:1], tile_r0 * ROWE + do, [[ROWE, n], [1, 3]]),
                    in_=_dview(img_a, base + so, [[0, 1], [rs, n], [1, 3]]),
                )
            nc.sync.dma_start(
                out=_fview(ta[127:128], tile_r0 * ROWE, [[ROWE, n], [1, 54]]),
                in_=_dview(img_a, base + 6090, [[0, 1], [rs, n], [1, 54]]),
            )
            for do, so in ((54, 6138), (57, 6135)):
                nc.sync.dma_start(
                    out=_fview(ta[127:128], tile_r0 * ROWE + do, [[ROWE, n], [1, 3]]),
                    in_=_dview(img_a, base + so, [[0, 1], [rs, n], [1, 3]]),
                )

        blocks = {}

        def emit_loads(b):
            r0 = b * r
            tin = tpool.tile([128, (r + 4) * ROWE], F32)
            ta = tin[:]
            if nb_blocks == 1:
                load_rows(ta, 2, r, 0, 1)
                load_rows(ta, 0, 2, 2, -1)
                load_rows(ta, r + 2, 2, h - 2, -1)
            elif b == 0:
                load_rows(ta, 2, r + 2, 0, 1)
                load_rows(ta, 0, 2, 2, -1)
            elif b == nb_blocks - 1:
                load_rows(ta, 0, r + 2, r0 - 2, 1)
                load_rows(ta, r + 2, 2, h - 2, -1)
            else:
                load_rows(ta, 0, r + 4, r0 - 2, 1)
            # planar fp16 copy of the pixel block: subs at 2x, ctr-add at 2x
            t16 = t16pool.tile([128, (r + 4) * ROWE], F16)
            t16a = t16[:]
            nc.scalar.activation(
                _fview(t16a, 0, [[ROWE, r + 4], [TCOL, 3], [1, 20]]),
                _fview(ta, 0, [[ROWE, r + 4], [1, 3], [3, 20]]),
                mybir.ActivationFunctionType.Copy)
            blocks[b] = (ta, t16a)

        def front(b, i, aa_ps):
            """sub / abs / t / sq / exp for pair i of block b."""
            dy, dx = PAIRS[i]
            ta, t16a = blocks[b]
            qr0 = -dy
            nqr = r + dy
            qc0 = -max(dx, 0)
            col_lo = qc0 + 2
            col_e = col_lo & ~1
            nqc = 16 + abs(dx) + (col_lo - col_e)
            ri0 = qr0 + 2

            dt_ = dpool.tile([128, (r + 2) * ROWE], F16)
            ad_ = apool.tile([128, (r + 2) * ROWE], F16)
            tt_ = ttpool.tile([128, (r + 2) * TCOL], F16)
            tw_ = wpool.tile([128, (r + 2) * TCOL], F16)
            dv, av, tv, wv = dt_[:], ad_[:], tt_[:], tw_[:]

            d_out = _fview(dv, ri0 * ROWE + col_e,
                           [[ROWE, nqr], [TCOL, 3], [1, nqc]])
            if (dy, dx) in F32_SUBS:
                nc.vector.tensor_tensor(
                    d_out,
                    _fview(ta, (ri0 + dy) * ROWE + (col_e + dx) * 3,
                           [[ROWE, nqr], [1, 3], [3, nqc]]),
                    _fview(ta, ri0 * ROWE + col_e * 3,
                           [[ROWE, nqr], [1, 3], [3, nqc]]),
                    mybir.AluOpType.subtract)
            else:
                nc.vector.tensor_tensor(
                    d_out,
                    _fview(t16a, (ri0 + dy) * ROWE + col_e + dx,
                           [[ROWE, nqr], [TCOL, 3], [1, nqc]]),
                    _fview(t16a, ri0 * ROWE + col_e,
                           [[ROWE, nqr], [TCOL, 3], [1, nqc]]),
                    mybir.AluOpType.subtract)
            a_out = _fview(av, ri0 * ROWE + col_e,
                           [[ROWE, nqr], [TCOL, 3], [1, nqc]])
            if (dy, dx) in ABS_ON_DVE:
                # |x| on fp16 = clear the sign bit; int16 view keeps the
                # 2-byte packed 4x DVE mode
                nc.vector.tensor_scalar(
                    a_out.bitcast(mybir.dt.int16),
                    d_out.bitcast(mybir.dt.int16),
                    0x7FFF, None, mybir.AluOpType.bitwise_and)
            else:
                nc.scalar.activation(
                    a_out, d_out, mybir.ActivationFunctionType.Abs)
            tq = _fview(tv, ri0 * TCOL + col_e, [[TCOL, nqr], [1, nqc]])
            nc.vector.tensor_tensor(
                tq,
                _fview(av, ri0 * ROWE + 0 * TCOL + col_e,
                       [[ROWE, nqr], [1, nqc]]),
                _fview(av, ri0 * ROWE + 1 * TCOL + col_e,
                       [[ROWE, nqr], [1, nqc]]),
                mybir.AluOpType.add)
            nc.vector.tensor_tensor(
                tq, tq,
                _fview(av, ri0 * ROWE + 2 * TCOL + col_e,
                       [[ROWE, nqr], [1, nqc]]),
                mybir.AluOpType.add)
            wq = _fview(wv, ri0 * TCOL + col_e, [[TCOL, nqr], [1, nqc]])
            if (dy, dx) in SQ_ON_DVE:
                # u = t*t on DVE; fold a^2 into the exp scale
                nc.vector.tensor_tensor(tq, tq, tq, mybir.AluOpType.mult)
                nc.scalar.activation(wq, tq,
                                     mybir.ActivationFunctionType.Exp,
                                     bias=ca[:, 2 + i:3 + i],
                                     scale=float(INV2SR2))
            else:
                nc.scalar.activation(tq, tq,
                                     mybir.ActivationFunctionType.Square,
                                     scale=a_sq)
                nc.scalar.activation(wq, tq,
                                     mybir.ActivationFunctionType.Exp,
                                     bias=ca[:, 2 + i:3 + i], scale=neg1)
            return dict(dy=dy, dx=dx, dv=dv, wv=wv, ri0=ri0, col_e=col_e,
                        nqr=nqr, nqc=nqc, aa_ps=aa_ps)

        dir_state = [0]

        def back(st):
            """pi products + PE accumulation for a pair (emitted one pair
            late so the DVE never waits in-order on this pair's exp)."""
            dy, dx, dv, wv = st["dy"], st["dx"], st["dv"], st["wv"]
            ri0, col_e, nqr, nqc = st["ri0"], st["col_e"], st["nqr"], st["nqc"]
            aa_ps = st["aa_ps"]
            dq = _fview(dv, ri0 * ROWE + col_e,
                        [[ROWE, nqr], [TCOL, 3], [1, nqc]])
            nc.vector.tensor_tensor(
                dq, dq,
                _fview(wv, ri0 * TCOL + col_e, [[TCOL, nqr], [0, 3], [1, nqc]]),
                mybir.AluOpType.mult)
            # PE accumulation: resid += pi(p); resid -= pi(p-o);
            #                  den += w(p) + w(p-o)
            rc_pos = (2, 2)
            rc_neg = (2 - dy, 2 - dx)
            st0 = dir_state[0] == 0
            sp = dir_state[0] == NDIR - 2
            for j in range(r // 8):
                nc.tensor.matmul(
                    _fview(aa_ps, (8 * j) * 64, [[64, 8], [16, 3], [1, 16]]),
                    posIa,
                    _fview(dv, (rc_pos[0] + 8 * j) * ROWE + rc_pos[1],
                           [[ROWE, 8], [TCOL, 3], [1, 16]]),
                    start=st0, stop=False)
            for (ri2, ci2) in (rc_pos, rc_neg):
                for j in range(r // 32):
                    nc.tensor.matmul(
                        _fview(aa_ps, (32 * j) * 64 + 48,
                               [[64, 32], [1, 16]]),
                        posIa,
                        _fview(wv, (ri2 + 32 * j) * TCOL + ci2,
                               [[TCOL, 32], [1, 16]]),
                        start=False, stop=False)
            for j in range(r // 8):
                nc.tensor.matmul(
                    _fview(aa_ps, (8 * j) * 64, [[64, 8], [16, 3], [1, 16]]),
                    negIa,
                    _fview(dv, (rc_neg[0] + 8 * j) * ROWE + rc_neg[1],
                           [[ROWE, 8], [TCOL, 3], [1, 16]]),
                    start=False, stop=sp)
            dir_state[0] += 2

        emit_loads(0)
        for b in range(nb_blocks):
            r0 = b * r
            ta, t16a = blocks[b]
            # PSUM rows are 64 fp32 wide: [48 resid | 16 den]. Each 8-row
            # resid chunk is then exactly one 2048B PSUM zero-region, so
            # start_tensor_calc's region-granular zeroing stays aligned;
            # the den stripes ride in the same regions (no separate start).
            acc = pspool.tile([128, r * 64], F32)
            aa_ps = acc[:]
            dir_state[0] = 0
            prev = None
            for i in range(len(PAIRS)):
                f = front(b, i, aa_ps)
                if prev is not None:
                    back(prev)
                prev = f
            back(prev)
            if b + 1 < nb_blocks:
                emit_loads(b + 1)

            # Final combine: out = clip(ctr + resid * (1/(den+c)), 0, 1)
            rs_ = rspool.tile([128, r * 48], F16)
            rsa = rs_[:]
            nc.scalar.activation(
                _fview(rsa, 0, [[48, r], [16, 3], [1, 16]]),
                _fview(aa_ps, 0, [[64, r], [16, 3], [1, 16]]),
                mybir.ActivationFunctionType.Copy)
            r32 = rcpool.tile([128, r * 16], F32)
            rca = r32[:]
            nc.scalar.activation(
                rca, _fview(aa_ps, 48, [[64, r], [1, 16]]),
                mybir.ActivationFunctionType.Ln, bias=den_c)
            rr_ = rrpool.tile([128, r * 16], F16)
            rra = rr_[:]
            nc.scalar.activation(rra, rca, mybir.ActivationFunctionType.Exp,
                                 scale=neg1)
            spl_ = sppool.tile([128, r * 48], F16)
            spla = spl_[:]
            nc.vector.tensor_tensor(
                _fview(spla, 0, [[48, r], [16, 3], [1, 16]]),
                _fview(rsa, 0, [[48, r], [16, 3], [1, 16]]),
                _fview(rra, 0, [[16, r], [0, 3], [1, 16]]),
                mybir.AluOpType.mult)
            # center add in planar fp16 (2x), then one Act copy does
            # planar->interleaved AND fp16->fp32
            nc.vector.tensor_tensor(
                _fview(spla, 0, [[48, r], [16, 3], [1, 16]]),
                _fview(spla, 0, [[48, r], [16, 3], [1, 16]]),
                _fview(t16a, 2 * ROWE + 2, [[ROWE, r], [TCOL, 3], [1, 16]]),
                mybir.AluOpType.add)
            stage = gpool.tile([128, r * 48], F32)
            sa = stage[:]
            nc.scalar.activation(
                _fview(sa, 0, [[48, r], [1, 3], [3, 16]]),
                _fview(spla, 0, [[48, r], [16, 3], [1, 16]]),
                mybir.ActivationFunctionType.Copy)
            nc.vector.tensor_scalar(sa, sa, 0.0, 1.0,
                                    mybir.AluOpType.max, mybir.AluOpType.min)
            nc.sync.dma_start(
                out=_dview(out_a, r0 * rowlen, [[48, 128], [rowlen, r], [1, 48]]),
                in_=_fview(sa, 0, [[48, r], [1, 48]]),
            )
    nc.finalize()
    return nc


_CACHE = {}


def _get_nc(h=H, r=R):
    key = (h, r)
    if key not in _CACHE:
        _CACHE[key] = build_nc(h, r)
    return _CACHE[key]


TRACE = False
LAST_RESULT = None


def kernel(images: np.ndarray) -> np.ndarray:
    global LAST_RESULT
    assert images.shape == (NCORES, H, W, C), images.shape
    nc = _get_nc()
    in_maps = [{"images": np.ascontiguousarray(images[i], dtype=np.float32)}
               for i in range(NCORES)]
    res = run_bass_kernel_spmd(nc, in_maps, core_ids=list(range(NCORES)),
                               trace=TRACE)
    LAST_RESULT = res
    return np.stack([res.results[i]["out"] for i in range(NCORES)], axis=0)
